# revision 68
# baseline (speedup 1.0000x reference)
"""Trainium2 Bass kernel for nn_Block_54382875902076 (dense transformer block).

Reference computation (B=4, S=2048, E=512, H=8, D=64, fp32):
    res = x
    h   = LN1(x)                      (no bias, eps=1e-6)
    h   = res + Attn(h)               (causal, wo1 [H,D,E] then wo2 [E,E])
    h   = LN2(h)
    out = res + gelu(h @ w1) @ w2     (NOTE: res = block input, both residuals)

Sharding (8 cores): core c = (batch b = c//2, head-group g = c%2).
Each core computes LN1 + QKV for its 4 heads over the full sequence,
exact-causal attention, a TRANSPOSED wo1 partial projection (o1T [E,q]),
pair-wise ReduceScatters deliver each core's own 256-row shard already
E-major (no post-RS transposes), then wo2 + LN2 + MLP per 256-row block.

v2 restructure (262us baseline -> 233us measured):
 - ONE software-pipelined loop: QKV(qt+1), o1rT load, wo1+RS(qt-1) and
   wo2/LN2(qt-2) are emitted interleaved with attention(qt)'s chunks.
   The AV matmuls lag the scores/exp by 2 chunks and the lag CROSSES
   group boundaries so the ACT exp stream never breaks; the AV psum
   spill to SBUF releases the psum slots early and feeds the
   reciprocal chain (scatter-DMA [16,64] recip, PE K=1 ones-matmul
   broadcast, psum*sbuf multiplies into fp8 attnT).
 - fp8e4m3 DoubleRow matmuls for QKV/wo1/wo2 (weights 16x-scaled on
   host; xq and w2 16x too so both residual adds agree; host divides
   the output by 16; exp scale absorbs 1/256).  m1/m2 stay bf16 --
   measured on CPU, fp8 there alone costs +2.1e-2 L2 (signal and
   quantization noise both grow as sqrt(K), so the error does NOT
   average out); qkv/av/wo1/wo2 cost only ~3-5e-3 each.
 - exact-causal column shrink: score MMs, exp, and AV accumulation
   skip fully-masked columns of diagonal chunks; the remaining
   triangular strip is two [128,128] multiplies per diagonal chunk.
 - MLP (gelu) runs in a PE-dense tail: gelu's activation-table set
   cannot interleave with exp's (one ACT_TABLE_LOAD costs ~2.7us and
   there is no table set holding both), so mlp1/mlp2 for all four
   256-row blocks run after the last exp, 4 psum slots deep.
 - wo1 emits o1T (lhsT=wo1): the ReduceScatter shards arrive E-major,
   so no post-RS transposes; RS input is split-stored per shard.
 - two tiny warmup ReduceScatters in the preamble absorb the ~50us
   first-collective init; collectives own the gpsimd queue exclusively
   (a collective dispatch blocks its issuing queue for 5-10us).
 - emission order IS dependency order for Tile: attn(qt) force-drains
   the qkv(qt) generator before reading KT/QT/V65, and wo1(qt) flushes
   any deferred AV/den work for its block before reading attnT.
"""

import functools
import sys

import numpy as np

for _p in ("/opt/trn_rl_repo", "/root/.axon_site/_ro/trn_rl_repo"):
    if _p not in sys.path:
        sys.path.append(_p)

import ml_dtypes  # noqa: E402
import concourse.bass as bass  # noqa: E402
import concourse.tile as tile  # noqa: E402
from concourse import bacc, mybir  # noqa: E402
from concourse.bass_utils import run_bass_kernel_spmd  # noqa: E402

_ALLOWED_ACT_SETS = {"natural_log_exp_and_others", "gelu_apprx_tanh_and_others"}
_orig_get_act_tables = bacc.get_activation_tables


def _filtered_act_tables(module_arch):
    tabs = _orig_get_act_tables(module_arch)
    return {
        name: (funcs if name in _ALLOWED_ACT_SETS else set())
        for name, funcs in tabs.items()
    }


bacc.get_activation_tables = _filtered_act_tables

F32 = mybir.dt.float32
BF16 = mybir.dt.bfloat16
F8 = mybir.dt.float8e4
AF = mybir.ActivationFunctionType
ALU = mybir.AluOpType

B, S, E, H, D = 4, 2048, 512, 8, 64
HG = H // 2            # heads per core
SQ = S // 2            # rows per core after reduce-scatter
NT = S // 128          # 16 token tiles (full seq)
NTQ = SQ // 128        # 8 token tiles (own half)
QTS = S // 512         # 4 q-tiles of 512 for attention


def _build_graph():
    nc = bacc.Bacc("TRN2", target_bir_lowering=False, debug=False, num_devices=8)

    xf = nc.declare_dram_parameter("xf", [128, NT, E], BF16, isOutput=False)
    xq = nc.declare_dram_parameter("xq", [128, NTQ, E], F32, isOutput=False)
    # wq/wk/wv/wo1/wo2 are 16x-scaled fp8 (DoubleRow); w2 is 16x bf16;
    # xq is 16x so both residual adds stay consistent (host divides /16)
    wq = nc.declare_dram_parameter("wq", [128, 4, HG * D], F8, isOutput=False)
    wk = nc.declare_dram_parameter("wk", [128, 4, HG * D], F8, isOutput=False)
    wv = nc.declare_dram_parameter("wv", [128, 4, HG * D], F8, isOutput=False)
    wo1 = nc.declare_dram_parameter("wo1", [128, 2, E], F8, isOutput=False)
    wo2 = nc.declare_dram_parameter("wo2", [128, 4, E], F8, isOutput=False)
    w1 = nc.declare_dram_parameter("w1", [128, 4, 4 * E], BF16, isOutput=False)
    w2 = nc.declare_dram_parameter("w2", [128, 16, E], BF16, isOutput=False)
    tri = nc.declare_dram_parameter("tri", [128, 128], BF16, isOutput=False)
    out = nc.declare_dram_parameter("out", [SQ, E], F32, isOutput=True)

    with tile.TileContext(nc) as tc:
        with (
            tc.tile_pool(name="consts", bufs=1) as consts,
            tc.tile_pool(name="acts", bufs=1) as acts,
            tc.tile_pool(name="work", bufs=3) as work,
            tc.tile_pool(name="stats", bufs=4) as stats,
            tc.tile_pool(name="den", bufs=2) as den,
            tc.tile_pool(name="lnw", bufs=8) as lnw,
            tc.tile_pool(name="expp", bufs=5) as expp,
            tc.tile_pool(name="m1p", bufs=2) as m1p,
            tc.tile_pool(name="psA", bufs=2, space="PSUM") as psA,
            tc.tile_pool(name="psB", bufs=2, space="PSUM") as psB,
            tc.tile_pool(name="psC", bufs=2, space="PSUM") as psC,
            tc.tile_pool(name="dram", bufs=1, space="DRAM") as dram,
        ):
            # ---- constants / weights (contiguous loads, gpsimd queue) ----
            eps_t = consts.tile([128, 1], F32)
            nc.vector.memset(eps_t, 1e-6)
            # dummy exp: pull the natural_log_exp table load to t=0 so the
            # first real LN/exp doesn't pay the ~2.7us PSEUDO_LOAD
            warm_act = consts.tile([128, 1], F32, tag="warm_act")
            nc.scalar.activation(warm_act[:], eps_t[:], AF.Exp)
            ident = consts.tile([128, 128], BF16)
            from concourse.masks import make_identity
            make_identity(nc, ident[:])


            def load_const(shape, src, tag, dt=BF16):
                t = consts.tile(shape, dt, tag=tag)
                nc.gpsimd.dma_start(t[:], src[:])
                return t

            wq_sb = load_const([128, 4, HG * D], wq, "wq_sb", F8)
            wk_sb = load_const([128, 4, HG * D], wk, "wk_sb", F8)
            wv_sb = load_const([128, 4, HG * D], wv, "wv_sb", F8)
            tri_sb = load_const([128, 128], tri, "tri_sb")
            # loads not needed until attn(1)+ stream in during attn(0)
            wo1_sb = consts.tile([128, 2, E], F8, tag="wo1_sb")
            wo2_sb = consts.tile([128, 4, E], F8, tag="wo2_sb")
            w1_sb = consts.tile([128, 4, 4 * E], BF16, tag="w1_sb")
            w2_sb = consts.tile([128, 16, E], BF16, tag="w2_sb")
            ones1 = consts.tile([1, 64], BF16)
            nc.vector.memset(ones1, 1.0)
            xq_sb = acts.tile([128, NTQ, E], F32)

            def late_loads():
                nc.gpsimd.dma_start(wo1_sb[:], wo1[:])
                nc.gpsimd.dma_start(wo2_sb[:], wo2[:])
                yield
                nc.gpsimd.dma_start(xq_sb[:], xq[:])
                yield
                nc.gpsimd.dma_start(w1_sb[:], w1[:])
                yield
                nc.gpsimd.dma_start(w2_sb[:], w2[:])
                yield

            # warmup collective: absorbs first-collective init/skew (~50us)
            # during the preamble so RS(0) doesn't stall the pipeline
            cc_warm_in = dram.tile([2, 64], BF16, name="cc_warm_in")
            cc_warm_out = dram.tile([1, 64], BF16, name="cc_warm_out")
            warm_src = consts.tile([2, 64], BF16, tag="warm_src")
            nc.vector.memset(warm_src, 0.0)
            nc.sync.dma_start(cc_warm_in[:], warm_src[:])
            for _w in range(2):
                nc.gpsimd.collective_compute(
                    "ReduceScatter", ALU.add,
                    replica_groups=[[0, 1], [2, 3], [4, 5], [6, 7]],
                    ins=[cc_warm_in[:].opt()],
                    outs=[cc_warm_out[:].opt()],
                )
            xfsb = consts.tile([128, NT, E], BF16, tag="xfsb")
            for _st in range(QTS):
                nc.sync.dma_start(
                    xfsb[:, 4 * _st:4 * _st + 4, :], xf[:, 4 * _st:4 * _st + 4, :]
                )

            # ---- persistent activations ---------------------------------
            h1T = acts.tile([128, 4, S], F8)
            KT = acts.tile([128, 2, S], BF16)
            QT = acts.tile([128, 2, S], BF16)
            # V65 holds 16*V (wv is 16x); ones column = 16 so P*V/den is exact
            V65 = acts.tile([128, NT, HG, D + 1], BF16)
            nc.vector.memset(V65[:, :, :, D:D + 1], 16.0)
            attnT = acts.tile([128, 2, S], F8)
            o1rT = acts.tile([128, 4, SQ], BF16)
            o1rT8 = acts.tile([128, 4, SQ], F8)
            h2T = acts.tile([128, 4, SQ], BF16)
            m1T_tiles = [
                m1p.tile([128, 16, 256], BF16, tag="m1T", name=f"m1T{qt}")
                for qt in range(QTS)
            ]
            o1Tp_dram = [dram.tile([2, E, 256], BF16, name=f"o1Tp{qt}")
                         for qt in range(QTS)]
            o1rT_dram = [dram.tile([E, 256], BF16, name=f"o1rT{qt}")
                         for qt in range(QTS)]

            def batched_ln(src_aps, dst_tiles, nb):
                """LayerNorm nb tiles together: one Ln+Exp pair on [128,nb]."""
                mvb = stats.tile([128, 2, 4], F32, tag="mvb")
                for i in range(nb):
                    st6 = stats.tile([128, 6], F32, tag="st6")
                    nc.vector.bn_stats(st6[:], src_aps[i])
                    nc.vector.bn_aggr(mvb[:, :, i], st6[:])
                lnv = stats.tile([128, 4], F32, tag="lnv")
                rsig = stats.tile([128, 4], F32, tag="rsig")
                nc.scalar.activation(lnv[:, 0:nb], mvb[:, 1, 0:nb], AF.Ln,
                                     bias=eps_t[:])
                nc.scalar.activation(rsig[:, 0:nb], lnv[:, 0:nb], AF.Exp,
                                     scale=-0.5)
                for i in range(nb):
                    nc.vector.tensor_scalar(
                        dst_tiles[i][:], src_aps[i], mvb[:, 0, i:i + 1],
                        rsig[:, i:i + 1], op0=ALU.subtract, op1=ALU.mult,
                    )

            # ---- stage generators (each yields small work units) --------
            def qkv_block(st):
                """LN1 + transpose + K/Q/V for 512-token block st."""
                h1ts = []
                srcs = [xfsb[:, t, :] for t in range(4 * st, 4 * st + 4)]
                for t in range(4):
                    h1ts.append(lnw.tile([128, E], BF16, tag="lnt",
                                         name=f"h1t{st}_{t}"))
                batched_ln(srcs, h1ts, 4)
                yield
                for lt in range(4):
                    for kp in range(2):  # ko pairs -> one [128,256] psum
                        psT = psC.tile([128, 512], BF16, tag="psC",
                                       name=f"psH{st}_{lt}_{kp}")
                        for kk in range(2):
                            ko = 2 * kp + kk
                            nc.tensor.transpose(
                                psT[:, kk * 128:(kk + 1) * 128],
                                h1ts[lt][:, ko * 128:(ko + 1) * 128], ident[:]
                            )
                        col = st * 512 + lt * 128
                        nc.vector.tensor_copy(
                            h1T[:, 2 * kp:2 * kp + 2, col:col + 128],
                            psT[:, 0:256].rearrange("p (k c) -> p k c", k=2),
                        )
                    yield
                sl = slice(st * 512, (st + 1) * 512)
                DR = mybir.MatmulPerfMode.DoubleRow
                for mi in range(2):
                    for dst, w_sb in ((KT, wk_sb), (QT, wq_sb)):
                        ps = psC.tile([128, 512], F32, tag="psC")
                        for cc in range(2):
                            nc.tensor.matmul(
                                ps[:],
                                lhsT=w_sb[:, 2 * cc:2 * cc + 2,
                                          mi * 128:(mi + 1) * 128],
                                rhs=h1T[:, 2 * cc:2 * cc + 2, sl],
                                start=(cc == 0), stop=(cc == 1),
                                perf_mode=DR,
                            )
                        nc.vector.tensor_copy(dst[:, mi, sl], ps[:])
                        yield
                for tt in range(4 * st, 4 * st + 4):
                    ps = psC.tile([128, 512], F32, tag="psC")
                    for cc in range(2):
                        nc.tensor.matmul(
                            ps[:, 0:HG * D],
                            lhsT=h1T[:, 2 * cc:2 * cc + 2,
                                     tt * 128:(tt + 1) * 128],
                            rhs=wv_sb[:, 2 * cc:2 * cc + 2, :],
                            start=(cc == 0), stop=(cc == 1),
                            perf_mode=DR,
                        )
                    nc.vector.tensor_copy(
                        V65[:, tt, :, 0:D],
                        ps[:, 0:HG * D].rearrange("p (h d) -> p h d", h=HG),
                    )
                    yield

            def wo1_rs_block(qt):
                """Transposed wo1 partials + split-store + ReduceScatter."""
                qsl = slice(qt * 512, (qt + 1) * 512)
                # attnT(qt) must be fully written: force out any still-
                # deferred AV/den work for this block before reading it
                while avq and avq[0]["qt"] == qt:
                    pop_av()
                if pending_fin[0] is not None and pending_fin[0][0] == qt:
                    den_finish(*pending_fin[0])
                    pending_fin[0] = None
                for ec in range(4):
                    ps = psC.tile([128, 512], F32, tag="psC")
                    nc.tensor.matmul(
                        ps[:],
                        lhsT=wo1_sb[:, 0:2, ec * 128:(ec + 1) * 128],
                        rhs=attnT[:, 0:2, qsl],
                        start=True, stop=True,
                        perf_mode=mybir.MatmulPerfMode.DoubleRow,
                    )
                    o1t = work.tile([128, E], BF16, tag="wbf",
                                    name=f"o1t{qt}_{ec}")
                    nc.vector.tensor_scalar_mul(o1t[:], ps[:], 1.0 / 16.0)
                    for gg in range(2):
                        nc.sync.dma_start(
                            o1Tp_dram[qt][gg, ec * 128:(ec + 1) * 128, :],
                            o1t[:, gg * 256:(gg + 1) * 256],
                        )
                    yield
                nc.gpsimd.collective_compute(
                    "ReduceScatter", ALU.add,
                    replica_groups=[[0, 1], [2, 3], [4, 5], [6, 7]],
                    ins=[o1Tp_dram[qt][:].opt()],
                    outs=[o1rT_dram[qt][:].opt()],
                )
                yield

            def o1r_load(qt):
                sl = slice(qt * 256, (qt + 1) * 256)
                nc.sync.dma_start(
                    o1rT[:, :, sl],
                    o1rT_dram[qt][:].rearrange("(k p) t -> p k t", p=128),
                )
                yield
                with nc.allow_low_precision(reason="fp8 wo2 operand"):
                    nc.vector.tensor_copy(o1rT8[:, :, sl], o1rT[:, :, sl])
                yield

            def wo2_ln2_block(qt):
                """wo2 + residual + LN2 + h2 transpose for own 256 rows."""
                h2ts = []
                h2rs = []
                for tc in range(2):
                    ps = psC.tile([128, 512], F32, tag="psC")
                    for cc in range(2):
                        nc.tensor.matmul(
                            ps[:],
                            lhsT=o1rT8[:, 2 * cc:2 * cc + 2,
                                       qt * 256 + tc * 128:
                                       qt * 256 + (tc + 1) * 128],
                            rhs=wo2_sb[:, 2 * cc:2 * cc + 2, :],
                            start=(cc == 0), stop=(cc == 1),
                            perf_mode=mybir.MatmulPerfMode.DoubleRow,
                        )
                    h2r = work.tile([128, E], F32, tag="wf32",
                                    name=f"h2r{qt}_{tc}")
                    nc.vector.tensor_add(h2r[:], ps[:], xq_sb[:, 2 * qt + tc, :])
                    h2rs.append(h2r)
                    h2ts.append(lnw.tile([128, E], BF16, tag="lnt",
                                         name=f"h2t{qt}_{tc}"))
                    yield
                batched_ln([h2r[:] for h2r in h2rs], h2ts, 2)
                yield
                for tc in range(2):
                    for kp in range(2):
                        psT = psC.tile([128, 512], BF16, tag="psC",
                                       name=f"psG{qt}_{tc}_{kp}")
                        for kk in range(2):
                            ko = 2 * kp + kk
                            nc.tensor.transpose(
                                psT[:, kk * 128:(kk + 1) * 128],
                                h2ts[tc][:, ko * 128:(ko + 1) * 128], ident[:]
                            )
                        col = qt * 256 + tc * 128
                        nc.vector.tensor_copy(
                            h2T[:, 2 * kp:2 * kp + 2, col:col + 128],
                            psT[:, 0:256].rearrange("p (k c) -> p k c", k=2),
                        )
                        yield

            def mlp1_block(qt):
                """m1 = gelu(h2 @ w1), m-major [128m, 16mi, 256t].

                Runs in the tail when attention's psA pool is free -> use
                it so m1/m2 streams have 4 psum slots between them."""
                m1T = m1T_tiles[qt]
                hsl = slice(qt * 256, (qt + 1) * 256)
                for mp in range(8):
                    ps = psA.tile([128, 512], F32, tag="psA")
                    for half in range(2):
                        mi = 2 * mp + half
                        for ko in range(4):
                            nc.tensor.matmul(
                                ps[:, half * 256:(half + 1) * 256],
                                lhsT=w1_sb[:, ko, mi * 128:(mi + 1) * 128],
                                rhs=h2T[:, ko, hsl],
                                start=(ko == 0), stop=(ko == 3),
                            )
                    nc.scalar.activation(
                        m1T[:, 2 * mp:2 * mp + 2, :],
                        ps[:].rearrange("p (k c) -> p k c", k=2),
                        AF.Gelu_apprx_tanh,
                    )
                    yield

            def mlp2_block(qt):
                m1T = m1T_tiles[qt]
                for tc in range(2):
                    ps = psC.tile([128, 512], F32, tag="psC")
                    for ko in range(16):
                        nc.tensor.matmul(
                            ps[:],
                            lhsT=m1T[:, ko, tc * 128:(tc + 1) * 128],
                            rhs=w2_sb[:, ko, :],
                            start=(ko == 0), stop=(ko == 15),
                        )
                    ot = work.tile([128, E], F32, tag="wf32",
                                   name=f"ot{qt}_{tc}")
                    nc.vector.tensor_add(ot[:], ps[:], xq_sb[:, 2 * qt + tc, :])
                    nc.sync.dma_start(
                        out[(2 * qt + tc) * 128:(2 * qt + tc + 1) * 128, :],
                        ot[:])
                    yield

            # ---- attention -----------------------------------------------
            # Per head-pair group: scores+exp stream ahead, AV lags 2
            # chunks, and the previous group's denominator finish (DMA
            # round-trip + broadcast matmul + psum multiplies) is emitted
            # after this group's first two chunks so the PE/ACT queues
            # never sit behind the DMA chain.
            def den_spill(qt, a, avA, avB):
                """Copy AV psums to SBUF right away (frees psB for the next
                group's accumulation) and kick off the reciprocal chain."""
                avsA = den.tile([65, 512], BF16, tag="avsA",
                                name=f"avsA{qt}_{a}")
                avsB = den.tile([65, 512], BF16, tag="avsB",
                                name=f"avsB{qt}_{a}")
                nc.vector.tensor_copy(avsA[:], avA[0:65, :])
                nc.vector.tensor_copy(avsB[:], avB[0:65, :])
                d16 = den.tile([16, 64], BF16, tag="d16")
                nc.sync.dma_start(
                    d16[0:8, :],
                    avsA[64:65, :].rearrange("o (p f) -> o p f", p=8))
                nc.sync.dma_start(
                    d16[8:16, :],
                    avsB[64:65, :].rearrange("o (p f) -> o p f", p=8))
                r16 = den.tile([16, 64], BF16, tag="r16")
                with nc.allow_low_precision(reason="1/den broadcast in bf16"):
                    nc.vector.reciprocal(r16[:], d16[:])
                rr1 = den.tile([1, 1024], BF16, tag="rr1")
                nc.sync.dma_start(
                    rr1.rearrange("o (p f) -> o p f", p=16), r16[:]
                )
                return avsA, avsB, rr1

            def den_finish(qt, a, avsA, avsB, rr1):
                qsl = slice(qt * 512, (qt + 1) * 512)
                # PE K=1 ones-matmul broadcasts 1/den rows (gpsimd must stay
                # free for collectives -- its queue blocks ~6us per dispatch)
                # odd head (avsB, den in rr1 cols 512:1024) -> rows 0:64
                denbO = psC.tile([128, 512], F32, tag="psC",
                                 name=f"denbO{qt}_{a}")
                nc.tensor.matmul(denbO[0:64, :], lhsT=ones1[:],
                                 rhs=rr1[0:1, 512:1024], start=True, stop=True)
                with nc.allow_low_precision(reason="fp8 attnT"):
                    nc.vector.tensor_tensor(
                        attnT[0:64, a, qsl], avsB[0:64, :], denbO[0:64, :],
                        op=ALU.mult)
                # even head (avsA) -> tmp, DMA into attnT rows 64:128
                denbE = psC.tile([128, 512], F32, tag="psC",
                                 name=f"denbE{qt}_{a}")
                nc.tensor.matmul(denbE[0:64, :], lhsT=ones1[:],
                                 rhs=rr1[0:1, 0:512], start=True, stop=True)
                tmpE = work.tile([64, 512], F8, tag="atmp")
                with nc.allow_low_precision(reason="fp8 attnT"):
                    nc.vector.tensor_tensor(
                        tmpE[:], avsA[0:64, :], denbE[0:64, :], op=ALU.mult)
                nc.sync.dma_start(attnT[64:128, a, qsl], tmpE[:])

            pending_fin = [None]
            avq = []     # deferred AV emissions; the lag crosses group
                         # boundaries so the exp stream never breaks

            def pop_av():
                e = avq.pop(0)
                e["emit"]()
                if e["end"]:
                    g = e["grp"]
                    pending_fin[0] = (e["qt"], e["a"]) + den_spill(
                        e["qt"], e["a"], g["avA"], g["avB"])

            def attn_block(qt, trail):
                def pump(n):
                    for _ in range(n):
                        if not trail:
                            return
                        try:
                            next(trail[0][0])
                        except StopIteration:
                            trail.pop(0)

                # emission-order guarantee: all of qkv(qt)'s instructions
                # (KT/QT/V writes) must be EMITTED before attention reads
                # them -- Tile cannot dep-track reads emitted before writes
                while any(name == f"qkv{qt}" for _, name in trail):
                    pump(1)

                ext = 4 * (qt + 1)
                for a in range(2):
                    # psB slots are allocated lazily at the FIRST deferred-AV
                    # emission -- which is always after the previous group's
                    # den_spill reads, so slot reuse is safely ordered
                    grp = {"avA": None, "avB": None}

                    def alloc(qt=qt, a=a, grp=grp):
                        if grp["avA"] is None:
                            grp["avA"] = psB.tile([128, 512], F32, tag="psB",
                                                  name=f"avA{qt}_{a}")
                            grp["avB"] = psB.tile([128, 512], F32, tag="psB",
                                                  name=f"avB{qt}_{a}")
                        return grp["avA"], grp["avB"]

                    for c in range(ext):
                        j = c - 4 * qt
                        left = max(0, 128 * j)
                        sp = psA.tile([128, 1024], F32, tag="psA")
                        sp3 = sp.rearrange("p (h q) -> p h q", h=2)
                        nc.tensor.matmul(
                            sp[:, left:512],
                            lhsT=KT[0:64, a, c * 128:(c + 1) * 128],
                            rhs=QT[0:64, a, qt * 512 + left:(qt + 1) * 512],
                            start=True, stop=True,
                        )
                        nc.tensor.matmul(
                            sp[:, 512 + left:1024],
                            lhsT=KT[64:128, a, c * 128:(c + 1) * 128],
                            rhs=QT[64:128, a, qt * 512 + left:(qt + 1) * 512],
                            start=True, stop=True,
                        )
                        ex = expp.tile([128, 2, 512], BF16, tag="ex")
                        nc.scalar.activation(
                            ex[:, :, left:512], sp3[:, :, left:512],
                            AF.Exp, scale=D ** -0.5 / 256.0)
                        if j >= 0:
                            for hh in range(2):
                                nc.vector.tensor_tensor(
                                    ex[:, hh, left:left + 128],
                                    ex[:, hh, left:left + 128],
                                    tri_sb[:], op=ALU.mult,
                                )

                        def em(c=c, left=left, ex=ex, alloc=alloc,
                               a=a, first=(c == 0), last=(c == ext - 1)):
                            avA, avB = alloc()
                            nc.tensor.matmul(
                                avA[0:65, left:512],
                                lhsT=V65[:, c, 2 * a, :],
                                rhs=ex[:, 0, left:512],
                                start=first, stop=last,
                            )
                            nc.tensor.matmul(
                                avB[0:65, left:512],
                                lhsT=V65[:, c, 2 * a + 1, :],
                                rhs=ex[:, 1, left:512],
                                start=first, stop=last,
                            )

                        avq.append({"emit": em, "end": c == ext - 1,
                                    "qt": qt, "a": a, "grp": grp})
                        while len(avq) > 2:
                            pop_av()
                        if c == 3 and pending_fin[0] is not None:
                            den_finish(*pending_fin[0])
                            pending_fin[0] = None
                        pump(1)

            # ---- the pipelined schedule ---------------------------------
            for _ in qkv_block(0):
                pass
            trail = []
            for qt in range(QTS):
                if qt == 0:
                    trail.append((late_loads(), "loads"))
                if qt + 1 < QTS:
                    trail.append((qkv_block(qt + 1), f"qkv{qt + 1}"))
                if qt >= 2:
                    trail.append((o1r_load(qt - 2), f"o1r{qt - 2}"))
                if qt >= 1:
                    trail.append((wo1_rs_block(qt - 1), f"wo1{qt - 1}"))
                if qt >= 2:
                    trail.append((wo2_ln2_block(qt - 2), f"wo2{qt - 2}"))
                if qt == QTS - 1:
                    # RS(2) finishes mid-attn(3): pull block 2's wo2/LN2
                    # into the attention span (exp table still resident)
                    trail.append((o1r_load(QTS - 2), f"o1r{QTS - 2}"))
                    trail.append((wo2_ln2_block(QTS - 2), f"wo2{QTS - 2}"))
                attn_block(qt, trail)
            while avq:
                pop_av()
            for gen, _name in trail:    # drain leftovers
                for _ in gen:
                    pass
            if pending_fin[0] is not None:
                den_finish(*pending_fin[0])
                pending_fin[0] = None
            # tail: last wo1+RS, then the gelu-table MLP for all blocks
            # (gelu's activation-table set must not interleave with exp's)
            # everything not needing RS(3) runs BEFORE the o1r_load(3)/
            # wo2(3) chain so the in-order PE queue never sits behind it
            for gen in [wo1_rs_block(QTS - 1), mlp1_block(0),
                        mlp1_block(1), mlp2_block(0),
                        mlp1_block(2), mlp2_block(1), mlp2_block(2),
                        o1r_load(QTS - 1), wo2_ln2_block(QTS - 1),
                        mlp1_block(3), mlp2_block(3)]:
                for _ in gen:
                    pass

    nc.finalize()
    return nc


@functools.lru_cache(maxsize=1)
def _get_graph():
    return _build_graph()


def _bf16_kpm(a, p=128):
    """[K, M] fp32 -> contiguous [p, K//p, M] bf16 (SBUF (k p) layout)."""
    k, m = a.shape
    return np.ascontiguousarray(
        a.reshape(k // p, p, m).transpose(1, 0, 2)
    ).astype(ml_dtypes.bfloat16)


def _f8_kpm(a, p=128):
    """16x-scaled fp8e4m3 version of _bf16_kpm (for DoubleRow matmuls)."""
    k, m = a.shape
    return np.ascontiguousarray(
        (16.0 * a).reshape(k // p, p, m).transpose(1, 0, 2)
    ).astype(ml_dtypes.float8_e4m3)


def _own_rows(rank):
    """Global row indices owned by a core after the per-block reduce-scatters."""
    return np.concatenate(
        [np.arange(512 * qt + 256 * rank, 512 * qt + 256 * rank + 256) for qt in range(QTS)]
    )


def _make_in_maps(x, wq, wk, wv, wo1, wo2, w1, w2, ln1_scale, ln2_scale):
    x = np.asarray(x, dtype=np.float32)
    wq = np.asarray(wq, dtype=np.float32).reshape(E, H * D)
    wk = np.asarray(wk, dtype=np.float32).reshape(E, H * D)
    wv = np.asarray(wv, dtype=np.float32).reshape(E, H * D)
    wo1 = np.asarray(wo1, dtype=np.float32).reshape(H * D, E)
    wo2 = np.asarray(wo2, dtype=np.float32)
    w1 = np.asarray(w1, dtype=np.float32)
    w2 = np.asarray(w2, dtype=np.float32)
    s1 = np.asarray(ln1_scale, dtype=np.float32)[:, None]
    s2 = np.asarray(ln2_scale, dtype=np.float32)[:, None]

    wq_s, wk_s, wv_s = s1 * wq, s1 * wk, s1 * wv
    w1_s = s2 * w1

    # one shared triangular strip mask: tri[p, f] = 1.0 iff p <= f
    iota = np.arange(128)
    tri_np = (iota[:, None] <= iota[None, :]).astype(np.float32)
    tri_np = np.ascontiguousarray(tri_np).astype(ml_dtypes.bfloat16)

    in_maps = []
    for c in range(8):
        b, g = c // 2, c % 2
        hd = slice(g * HG * D, (g + 1) * HG * D)
        rows = _own_rows(c % 2)
        xq_arr = np.ascontiguousarray(
            16.0 * x[b][rows].reshape(NTQ, 128, E).transpose(1, 0, 2)
        )
        # attnT puts the ODD head of each pair on partitions 0:64 and the
        # EVEN head on 64:128 -> permute wo1's hd rows to match.
        perm = np.arange(HG * D).reshape(HG, D)
        perm = perm.reshape(2, 2, D)[:, ::-1, :].reshape(-1)
        wo1_c = wo1[hd, :][perm, :]
        in_maps.append({
            "xf": np.ascontiguousarray(x[b].reshape(NT, 128, E).transpose(1, 0, 2)).astype(ml_dtypes.bfloat16),
            "xq": xq_arr,
            "wq": _f8_kpm(wq_s[:, hd]),
            "wk": _f8_kpm(wk_s[:, hd]),
            "wv": _f8_kpm(wv_s[:, hd]),
            "wo1": _f8_kpm(wo1_c),
            "wo2": _f8_kpm(wo2),
            "w1": _bf16_kpm(w1_s),
            "w2": _bf16_kpm(16.0 * w2),
            "tri": tri_np,
        })
    return in_maps


def run(trace=False, **inputs):
    nc = _get_graph()
    in_maps = _make_in_maps(**inputs)
    res = run_bass_kernel_spmd(nc, in_maps, core_ids=list(range(8)), trace=trace)
    y = np.empty((B, S, E), dtype=np.float32)
    for c in range(8):
        b = c // 2
        y[b][_own_rows(c % 2)] = res.results[c]["out"]
    y *= 1.0 / 16.0   # kernel computes 16*(out) (16x weights/xq scaling)
    return y, res


def kernel(**inputs):
    y, _ = run(trace=False, **inputs)
    return y
